# revision 14
# baseline (speedup 1.0000x reference)
"""Trainium2 Bass kernel for the GRU + intent-net + masked-attention model.

Sharding: pure data-parallel over the batch dim across 8 NeuronCores.
BatchNorm (training mode) needs global batch stats -> in-kernel AllReduce
of per-core (sum, sumsq) partial stats (tiny, [128,4] fp32).

Device layout: activations are kept transposed ("T-layout", features on
partitions, batch*agents on the free axis) so every Linear maps directly
onto the PE array (contraction dim on partitions for both operands).
Matmuls run in float32r (full PE speed at moving-dim >= 256, ~1.5e-4 rel
error vs 2.3e-3 for bf16).
"""

import numpy as np

import concourse.bacc as bacc
import concourse.tile as tile
import concourse.mybir as mybir
from concourse.bass_utils import run_bass_kernel_spmd

F32 = mybir.dt.float32
F32R = mybir.dt.float32r
AX = mybir.AxisListType
OP = mybir.AluOpType
AF = mybir.ActivationFunctionType

B, N, IN, H, I, A, NA, NH = 4096, 8, 256, 256, 64, 64, 32, 256
VAR_FLOOR = 0.002
BN_EPS = 1e-5
NCORES = 8
R = B * N // NCORES          # 4096 rows per core
CHUNK = 512
NCH = R // CHUNK             # 8 chunks
NSUB = CHUNK // 128          # 4 sub-chunks per chunk

_CACHE = {}


def _build(collective=True):
    nc = bacc.Bacc("TRN2", target_bir_lowering=False)

    # ---- DRAM I/O ----
    xin = nc.dram_tensor("xin", [IN, R], F32R, kind="ExternalInput")
    hin = nc.dram_tensor("hin", [H, R], F32R, kind="ExternalInput")
    lat = nc.dram_tensor("lat", [NA, R], F32R, kind="ExternalInput")
    epst = nc.dram_tensor("epst", [I, R], F32, kind="ExternalInput")
    wfc1 = nc.dram_tensor("wfc1", [IN, H], F32R, kind="ExternalInput")
    wih = nc.dram_tensor("wih", [H, 3 * H], F32R, kind="ExternalInput")
    whh = nc.dram_tensor("whh", [H, 3 * H], F32R, kind="ExternalInput")
    win1 = nc.dram_tensor("win1", [H + NA, NH], F32R, kind="ExternalInput")
    win2 = nc.dram_tensor("win2", [NH, 2 * I], F32R, kind="ExternalInput")
    wqk = nc.dram_tensor("wqk", [I, 2 * A], F32R, kind="ExternalInput")
    wvw = nc.dram_tensor("wvw", [I + H, A], F32R, kind="ExternalInput")
    wfc2 = nc.dram_tensor("wfc2", [H + A, NA], F32R, kind="ExternalInput")
    fc2br = nc.dram_tensor("fc2br", [1, NA], F32R, kind="ExternalInput")
    cst = nc.dram_tensor("cst", [128, 20], F32, kind="ExternalInput")
    mskd = nc.dram_tensor("mskd", [128, 512], F32R, kind="ExternalInput")
    eyed = nc.dram_tensor("eyed", [128, 128], F32R, kind="ExternalInput")
    onesd = nc.dram_tensor("onesd", [128, 128], F32R, kind="ExternalInput")

    hT_o = nc.dram_tensor("hT_o", [H, R], F32, kind="ExternalOutput")
    ipT_o = nc.dram_tensor("ipT_o", [2 * I, R], F32, kind="ExternalOutput")
    intT_o = nc.dram_tensor("intT_o", [I, R], F32, kind="ExternalOutput")
    lq_o = nc.dram_tensor("lq_o", [R, NA], F32, kind="ExternalOutput")

    MM = nc.tensor.matmul
    ACT = nc.scalar.activation

    with tile.TileContext(nc) as tc:
        with (
            tc.tile_pool(name="wp", bufs=1) as wp,
            tc.tile_pool(name="res", bufs=1) as res,
            tc.tile_pool(name="pp", bufs=8, space="PSUM") as pp,
            tc.tile_pool(name="dp", bufs=1, space="DRAM") as dp,
        ):
            # ---- weights / constants into SBUF (once) ----
            w_fc1 = [wp.tile([128, H], F32R, tag=f"wfc1{k}", name=f"wfc1{k}") for k in range(2)]
            w_ih = [wp.tile([128, 3 * H], F32R, tag=f"wih{k}", name=f"wih{k}") for k in range(2)]
            w_hh = [wp.tile([128, 3 * H], F32R, tag=f"whh{k}", name=f"whh{k}") for k in range(2)]
            for k in range(2):
                nc.sync.dma_start(w_fc1[k][:], wfc1[k * 128:(k + 1) * 128, :])
                nc.sync.dma_start(w_ih[k][:], wih[k * 128:(k + 1) * 128, :])
                nc.sync.dma_start(w_hh[k][:], whh[k * 128:(k + 1) * 128, :])
            w_in1 = [wp.tile([128, NH], F32R, tag=f"win1{k}", name=f"win1{k}") for k in range(2)]
            w_in1.append(wp.tile([NA, NH], F32R, tag="win1la", name="win1la"))
            nc.sync.dma_start(w_in1[0][:], win1[0:128, :])
            nc.sync.dma_start(w_in1[1][:], win1[128:256, :])
            nc.sync.dma_start(w_in1[2][:], win1[256:288, :])
            w_in2 = [wp.tile([128, 2 * I], F32R, tag=f"win2{k}", name=f"win2{k}") for k in range(2)]
            for k in range(2):
                nc.sync.dma_start(w_in2[k][:], win2[k * 128:(k + 1) * 128, :])
            w_qk = wp.tile([I, 2 * A], F32R, tag="wqk", name="wqk")
            nc.sync.dma_start(w_qk[:], wqk[:])
            wv_i = wp.tile([I, A], F32R, tag="wvi", name="wvi")
            wv_h0 = wp.tile([128, A], F32R, tag="wvh0", name="wvh0")
            wv_h1 = wp.tile([128, A], F32R, tag="wvh1", name="wvh1")
            nc.sync.dma_start(wv_i[:], wvw[0:64, :])
            nc.sync.dma_start(wv_h0[:], wvw[64:192, :])
            nc.sync.dma_start(wv_h1[:], wvw[192:320, :])
            f2h0 = wp.tile([128, NA], F32R, tag="f2h0", name="f2h0")
            f2h1 = wp.tile([128, NA], F32R, tag="f2h1", name="f2h1")
            f2c = wp.tile([A, NA], F32R, tag="f2c", name="f2c")
            f2b = wp.tile([1, NA], F32R, tag="f2b", name="f2b")
            nc.sync.dma_start(f2h0[:], wfc2[0:128, :])
            nc.sync.dma_start(f2h1[:], wfc2[128:256, :])
            nc.sync.dma_start(f2c[:], wfc2[256:320, :])
            nc.sync.dma_start(f2b[:], fc2br[:])
            cs_t = wp.tile([128, 20], F32, tag="cst", name="cst")
            nc.sync.dma_start(cs_t[:], cst[:])
            msk = wp.tile([128, 512], F32R, tag="msk", name="msk")
            nc.sync.dma_start(msk[:], mskd[:])
            eye = wp.tile([128, 128], F32R, tag="eye", name="eye")
            nc.sync.dma_start(eye[:], eyed[:])
            ones128 = wp.tile([128, 128], F32R, tag="ones", name="ones")
            nc.sync.dma_start(ones128[:], onesd[:])

            # ---- residents ----
            hTs = res.tile([128, 2 * R], F32R, tag="hTs", name="hTs")    # [p, m*R + col]
            z1s = res.tile([128, 2 * R], F32R, tag="z1s", name="z1s")
            s1 = res.tile([128, 2 * NCH], F32, tag="s1", name="s1")     # col = m*NCH + c
            s2 = res.tile([128, 2 * NCH], F32, tag="s2", name="s2")
            gsc = res.tile([128, 2], F32, tag="gsc", name="gsc")         # g' = g*rsqrt(var+eps)
            gsh = res.tile([128, 2], F32, tag="gsh", name="gsh")         # b' = b - mean*g'

            # =========== PHASE A: fc1 -> GRU -> h ; in1 -> z1 + stats ===========
            with (
                tc.tile_pool(name="sda", bufs=2) as sda,
                tc.tile_pool(name="spa", bufs=2) as spa,
            ):
                for c in range(NCH):
                    cs = slice(c * CHUNK, (c + 1) * CHUNK)
                    xa = sda.tile([128, 2 * CHUNK], F32R, tag="xa", name="xa")
                    nc.sync.dma_start(
                        xa[:].rearrange("p (k x) -> p k x", k=2),
                        xin[:, cs].rearrange("(k p) x -> p k x", k=2),
                    )
                    ha = sda.tile([128, 2 * CHUNK], F32R, tag="ha", name="ha")
                    nc.sync.dma_start(
                        ha[:].rearrange("p (k x) -> p k x", k=2),
                        hin[:, cs].rearrange("(k p) x -> p k x", k=2),
                    )
                    la = sda.tile([NA, CHUNK], F32R, tag="la", name="la")
                    nc.sync.dma_start(la[:], lat[:, cs])

                    # fc1 + relu
                    xT = []
                    for m in range(2):
                        ms = slice(m * 128, (m + 1) * 128)
                        px = pp.tile([128, CHUNK], F32, tag="ps", name="ps")
                        MM(px[:], w_fc1[0][:, ms], xa[:, 0:CHUNK], start=True, stop=False)
                        MM(px[:], w_fc1[1][:, ms], xa[:, CHUNK:], start=False, stop=True)
                        xt = spa.tile([128, CHUNK], F32R, tag=f"xT{m}", name=f"xT{m}")
                        nc.vector.tensor_scalar(
                            xt[:], px[:], cs_t[:, m:m + 1], 0.0, OP.add, OP.max
                        )
                        xT.append(xt)

                    # gates: r, z accumulate gi+gh in psum
                    gate = {}
                    for gi, nm in enumerate(("r", "z")):
                        for m in range(2):
                            col = (gi * 2 + m) * 128
                            colsl = slice(col, col + 128)
                            pg = pp.tile([128, CHUNK], F32, tag="ps", name="ps")
                            MM(pg[:], w_ih[0][:, colsl], xT[0][:], start=True, stop=False)
                            MM(pg[:], w_ih[1][:, colsl], xT[1][:], start=False, stop=False)
                            MM(pg[:], w_hh[0][:, colsl], ha[:, 0:CHUNK], start=False, stop=False)
                            MM(pg[:], w_hh[1][:, colsl], ha[:, CHUNK:], start=False, stop=True)
                            g = spa.tile([128, CHUNK], F32, tag=f"{nm}{m}", name=f"{nm}{m}")
                            ACT(g[:], pg[:], AF.Sigmoid, bias=cs_t[:, 2 + gi * 2 + m:3 + gi * 2 + m])
                            gate[nm, m] = g

                    # n gate: i_n (psum), h_n (psum)
                    for m in range(2):
                        colsl = slice(512 + m * 128, 512 + (m + 1) * 128)
                        pni = pp.tile([128, CHUNK], F32, tag="ps", name="ps")
                        MM(pni[:], w_ih[0][:, colsl], xT[0][:], start=True, stop=False)
                        MM(pni[:], w_ih[1][:, colsl], xT[1][:], start=False, stop=True)
                        pnh = pp.tile([128, CHUNK], F32, tag="ps", name="ps")
                        MM(pnh[:], w_hh[0][:, colsl], ha[:, 0:CHUNK], start=True, stop=False)
                        MM(pnh[:], w_hh[1][:, colsl], ha[:, CHUNK:], start=False, stop=True)
                        hn = spa.tile([128, CHUNK], F32, tag=f"hn{m}", name=f"hn{m}")
                        ACT(hn[:], pnh[:], AF.Identity, bias=cs_t[:, 8 + m:9 + m])
                        t_ = spa.tile([128, CHUNK], F32, tag=f"t{m}", name=f"t{m}")
                        nc.gpsimd.tensor_tensor(t_[:], gate["r", m][:], hn[:], OP.mult)
                        u_ = spa.tile([128, CHUNK], F32, tag=f"u{m}", name=f"u{m}")
                        nc.vector.tensor_tensor(u_[:], pni[:], t_[:], OP.add)
                        nng = spa.tile([128, CHUNK], F32, tag=f"nng{m}", name=f"nng{m}")
                        ACT(nng[:], u_[:], AF.Tanh, bias=cs_t[:, 6 + m:7 + m])
                        d_ = spa.tile([128, CHUNK], F32, tag=f"d{m}", name=f"d{m}")
                        nc.gpsimd.tensor_tensor(
                            d_[:], ha[:, m * CHUNK:(m + 1) * CHUNK].bitcast(F32), nng[:], OP.subtract
                        )
                        e_ = spa.tile([128, CHUNK], F32, tag=f"e{m}", name=f"e{m}")
                        nc.vector.tensor_tensor(e_[:], gate["z", m][:], d_[:], OP.mult)
                        nc.vector.tensor_tensor(
                            hTs[:, m * R + c * CHUNK:m * R + (c + 1) * CHUNK], nng[:], e_[:], OP.add
                        )
                    nc.sync.dma_start(
                        hT_o[:, cs].rearrange("(m p) x -> p m x", m=2),
                        hTs[:].rearrange("p (m x) -> p m x", m=2)[:, :, cs].bitcast(F32),
                    )

                    # in1 -> z1 (+ stats)
                    for m in range(2):
                        ms = slice(m * 128, (m + 1) * 128)
                        pz = pp.tile([128, CHUNK], F32, tag="ps", name="ps")
                        MM(pz[:], w_in1[0][:, ms], hTs[:, 0 * R + c * CHUNK:0 * R + (c + 1) * CHUNK], start=True, stop=False)
                        MM(pz[:], w_in1[1][:, ms], hTs[:, 1 * R + c * CHUNK:1 * R + (c + 1) * CHUNK], start=False, stop=False)
                        MM(pz[:], w_in1[2][:, ms], la[:], start=False, stop=True)
                        z1sl = z1s[:, m * R + c * CHUNK:m * R + (c + 1) * CHUNK]
                        nc.vector.tensor_scalar(
                            z1sl, pz[:], cs_t[:, 10 + m:11 + m], 0.0, OP.add, OP.add,
                            accum_out=s1[:, m * NCH + c:m * NCH + c + 1],
                        )
                        sq = spa.tile([128, CHUNK], F32, tag="sq", name="sq")
                        ACT(
                            sq[:], z1sl.bitcast(F32), AF.Square,
                            accum_out=s2[:, m * NCH + c:m * NCH + c + 1],
                        )

            # =========== BN stats: reduce, AllReduce, scale/shift ===========
            st4 = res.tile([128, 4], F32, tag="st4", name="st4")
            for m in range(2):
                nc.vector.tensor_reduce(
                    st4[:, m:m + 1], s1[:, m * NCH:(m + 1) * NCH], AX.X, OP.add
                )
                nc.vector.tensor_reduce(
                    st4[:, 2 + m:3 + m], s2[:, m * NCH:(m + 1) * NCH], AX.X, OP.add
                )
            gstat = res.tile([128, 4], F32, tag="gstat", name="gstat")
            if collective:
                b_in = dp.tile([128, 4], F32, name="b_in")
                b_out = dp.tile([128, 4], F32)
                nc.gpsimd.dma_start(b_in[:], st4[:])
                nc.gpsimd.collective_compute(
                    "AllReduce", OP.add,
                    replica_groups=[list(range(NCORES))],
                    ins=[b_in[:].opt()], outs=[b_out[:].opt()],
                )
                nc.gpsimd.dma_start(gstat[:], b_out[:])
                n_total = float(NCORES * R)
            else:
                nc.vector.tensor_copy(gstat[:], st4[:])
                n_total = float(R)
            mn = res.tile([128, 2], F32, tag="mn", name="mn")
            nc.vector.tensor_scalar_mul(mn[:], gstat[:, 0:2], 1.0 / n_total)
            var = res.tile([128, 2], F32, tag="var", name="var")
            nc.vector.tensor_scalar_mul(var[:], gstat[:, 2:4], 1.0 / n_total)
            mn2 = res.tile([128, 2], F32, tag="mn2", name="mn2")
            nc.vector.tensor_tensor(mn2[:], mn[:], mn[:], OP.mult)
            nc.vector.tensor_tensor(var[:], var[:], mn2[:], OP.subtract)
            nc.vector.tensor_scalar_add(var[:], var[:], BN_EPS)
            rv = res.tile([128, 2], F32, tag="rv", name="rv")
            nc.vector.reciprocal(rv[:], var[:])
            rsq = res.tile([128, 2], F32, tag="rsq", name="rsq")
            ACT(rsq[:], rv[:], AF.Sqrt)
            nc.vector.tensor_tensor(gsc[:], rsq[:], cs_t[:, 12:14], OP.mult)
            tmp2 = res.tile([128, 2], F32, tag="tmp2", name="tmp2")
            nc.vector.tensor_tensor(tmp2[:], mn[:], gsc[:], OP.mult)
            nc.vector.tensor_tensor(gsh[:], cs_t[:, 14:16], tmp2[:], OP.subtract)

            # =========== PHASE B: BN-lrelu -> in2 -> intent -> attention -> fc2 ===========
            with (
                tc.tile_pool(name="sdb", bufs=2) as sdb,
                tc.tile_pool(name="spb", bufs=2) as spb,
            ):
                for c in range(NCH):
                    cs = slice(c * CHUNK, (c + 1) * CHUNK)
                    zb = []
                    for m in range(2):
                        z1sl = z1s[:, m * R + c * CHUNK:m * R + (c + 1) * CHUNK]
                        zt = spb.tile([128, CHUNK], F32R, tag=f"zb{m}", name=f"zb{m}")
                        ACT(zt[:], z1sl.bitcast(F32), AF.Prelu,
                            bias=gsh[:, m:m + 1], scale=gsc[:, m:m + 1], alpha=0.01)
                        zb.append(zt)

                    # in2 split into mean (out cols 0:64) and logstd (64:128)
                    # so both land at partition base 0 (lane-fixed engines).
                    pm = pp.tile([I, CHUNK], F32, tag="ps", name="ps")
                    MM(pm[:], w_in2[0][:, 0:64], zb[0][:], start=True, stop=False)
                    MM(pm[:], w_in2[1][:, 0:64], zb[1][:], start=False, stop=True)
                    pl = pp.tile([I, CHUNK], F32, tag="ps", name="ps")
                    MM(pl[:], w_in2[0][:, 64:128], zb[0][:], start=True, stop=False)
                    MM(pl[:], w_in2[1][:, 64:128], zb[1][:], start=False, stop=True)
                    mean_t = spb.tile([I, CHUNK], F32, tag="mean", name="mean")
                    nc.vector.tensor_scalar(
                        mean_t[:], pm[:], cs_t[0:64, 16:17], None, OP.add)
                    lgs_t = spb.tile([I, CHUNK], F32, tag="lgs", name="lgs")
                    nc.vector.tensor_scalar(
                        lgs_t[:], pl[:], cs_t[0:64, 18:19], None, OP.add)
                    nc.sync.dma_start(ipT_o[0:64, cs], mean_t[:])
                    nc.sync.dma_start(ipT_o[64:128, cs], lgs_t[:])
                    stdt = spb.tile([I, CHUNK], F32, tag="std", name="std")
                    ACT(stdt[:], pl[:], AF.Exp, bias=cs_t[0:64, 18:19])
                    nc.vector.tensor_scalar_max(stdt[:], stdt[:], VAR_FLOOR)
                    ept = sdb.tile([I, CHUNK], F32, tag="ept", name="ept")
                    nc.sync.dma_start(ept[:], epst[:, cs])
                    e1 = spb.tile([I, CHUNK], F32, tag="e1", name="e1")
                    nc.gpsimd.tensor_tensor(e1[:], ept[:], stdt[:], OP.mult)
                    itt = spb.tile([I, CHUNK], F32R, tag="itt", name="itt")
                    nc.vector.tensor_tensor(itt[:], e1[:], mean_t[:], OP.add)
                    nc.sync.dma_start(intT_o[:, cs], itt[:].bitcast(F32))

                    # q and k projections (separate base-0 psums)
                    pq = pp.tile([I, CHUNK], F32, tag="ps", name="ps")
                    MM(pq[:], w_qk[:, 0:64], itt[:], start=True, stop=True)
                    pk = pp.tile([I, CHUNK], F32, tag="ps", name="ps")
                    MM(pk[:], w_qk[:, 64:128], itt[:], start=True, stop=True)
                    qs_t = spb.tile([I, CHUNK], F32R, tag="qs", name="qs")
                    ACT(qs_t[:], pq[:], AF.Identity, bias=cs_t[0:64, 17:18])
                    ks_t = spb.tile([I, CHUNK], F32R, tag="ks", name="ks")
                    ACT(ks_t[:], pk[:], AF.Identity, bias=cs_t[0:64, 19:20])

                    # V (normal layout, 4 sub-chunks side by side)
                    pv = pp.tile([128, NSUB * A], F32, tag="ps", name="ps")
                    for s in range(NSUB):
                        ss = slice(s * 128, (s + 1) * 128)
                        asb = slice(c * CHUNK + s * 128, c * CHUNK + (s + 1) * 128)
                        vs = slice(s * A, (s + 1) * A)
                        MM(pv[:, vs], itt[:, ss], wv_i[:], start=True, stop=False)
                        MM(pv[:, vs], hTs[:, 0 * R + asb.start:0 * R + asb.stop], wv_h0[:], start=False, stop=False)
                        MM(pv[:, vs], hTs[:, 1 * R + asb.start:1 * R + asb.stop], wv_h1[:], start=False, stop=True)
                    vna = spb.tile([128, NSUB * 65], F32R, tag="vna", name="vna")
                    vv = vna[:].rearrange("p (s x) -> p s x", x=65)
                    nc.vector.tensor_copy(
                        vv[:, :, 64:65],
                        ones128[:, 0:NSUB].rearrange("p (s x) -> p s x", x=1))
                    ACT(vv[:, :, 0:64], pv[:].rearrange("p (s x) -> p s x", x=64), AF.Copy)

                    # S' = mask + K^T-major scores; Em' = exp
                    psc = pp.tile([128, CHUNK], F32, tag="ps", name="ps")
                    MM(psc[:], eye[:], msk[:], start=True, stop=False, skip_group_check=True)
                    for s in range(NSUB):
                        ss = slice(s * 128, (s + 1) * 128)
                        MM(psc[:, ss], ks_t[:, ss], qs_t[:, ss],
                           start=False, stop=(s == NSUB - 1), skip_group_check=True)
                    emt = spb.tile([128, CHUNK], F32R, tag="emt", name="emt")
                    ACT(emt[:], psc[:], AF.Exp)

                    # combined^T (unnormalized) + rowsums via ones column
                    pav = pp.tile([128, CHUNK], F32, tag="ps", name="ps")
                    for s in range(NSUB):
                        ss = slice(s * 128, (s + 1) * 128)
                        MM(pav[0:65, ss], vna[:, s * 65:(s + 1) * 65], emt[:, ss],
                           start=True, stop=True)
                    rstt = spb.tile([65, CHUNK], F32R, tag="rst", name="rst")
                    with nc.allow_low_precision(reason="softmax denom recip in f32r"):
                        nc.vector.reciprocal(rstt[64:65, :], pav[64:65, :])
                    pbc = pp.tile([128, CHUNK], F32, tag="ps", name="ps")
                    MM(pbc[0:64, :], ones128[64:65, 0:64], rstt[64:65, :],
                       start=True, stop=True)
                    avs = spb.tile([A, CHUNK], F32, tag="avs", name="avs")
                    ACT(avs[:], pav[0:64, :], AF.Copy)
                    comb = spb.tile([A, CHUNK], F32R, tag="comb", name="comb")
                    nc.vector.tensor_tensor(comb[:], avs[:], pbc[0:64, :], OP.mult)

                    # fc2 (normal layout out) with bias via ones-row matmul
                    pq = pp.tile([128, NSUB * NA], F32, tag="ps", name="ps")
                    for s in range(NSUB):
                        ss = slice(s * 128, (s + 1) * 128)
                        asb = slice(c * CHUNK + s * 128, c * CHUNK + (s + 1) * 128)
                        qs = slice(s * NA, (s + 1) * NA)
                        MM(pq[:, qs], hTs[:, 0 * R + asb.start:0 * R + asb.stop], f2h0[:], start=True, stop=False)
                        MM(pq[:, qs], hTs[:, 1 * R + asb.start:1 * R + asb.stop], f2h1[:], start=False, stop=False)
                        MM(pq[:, qs], comb[:, ss], f2c[:], start=False, stop=False)
                        MM(pq[:, qs], ones128[0:1, :], f2b[:], start=False, stop=True)
                    lqt = spb.tile([128, NSUB * NA], F32, tag="lqt", name="lqt")
                    nc.vector.tensor_copy(lqt[:], pq[:])
                    nc.sync.dma_start(
                        lq_o[cs, :].rearrange("(s p) x -> p s x", p=128),
                        lqt[:].rearrange("p (s x) -> p s x", x=NA),
                    )

    nc.compile()
    return nc


def _get_nc(collective=True):
    key = ("nc", collective)
    if key not in _CACHE:
        _CACHE[key] = _build(collective)
    return _CACHE[key]


def _prep_host(inputs, last_actions, hidden_state, eps,
               fc1_w, fc1_b, gru_w_ih, gru_w_hh, gru_b_ih, gru_b_hh,
               in1_w, in1_b, bn_g, bn_b, in2_w, in2_b,
               wq_w, wq_b, wk_w, wk_b, wv_w, wv_b, fc2_w, fc2_b):
    f = np.float32
    scale = f(1.0) / np.sqrt(f(A))

    shared = {
        "wfc1": np.ascontiguousarray(fc1_w.T, f),
        "wih": np.ascontiguousarray(gru_w_ih.T, f),
        "whh": np.ascontiguousarray(gru_w_hh.T, f),
        "win1": np.ascontiguousarray(in1_w.T, f),
        "win2": np.ascontiguousarray(in2_w.T, f),
        "wqk": np.ascontiguousarray(
            np.concatenate([wq_w.T * scale, wk_w.T], axis=1), f),
        "wvw": np.ascontiguousarray(wv_w.T, f),
        "wfc2": np.ascontiguousarray(fc2_w.T, f),
        "fc2br": np.ascontiguousarray(
            (fc2_b + fc2_w[:, H:] @ wv_b)[None, :], f),
    }

    cst = np.zeros((128, 20), f)
    def put2(col, vec):
        cst[:, col] = vec[0:128]
        cst[:, col + 1] = vec[128:256]
    put2(0, fc1_b)
    put2(2, gru_b_ih[0:256] + gru_b_hh[0:256])
    put2(4, gru_b_ih[256:512] + gru_b_hh[256:512])
    put2(6, gru_b_ih[512:768])
    put2(8, gru_b_hh[512:768])
    put2(10, in1_b)
    put2(12, bn_g)
    put2(14, bn_b)
    cst[0:64, 16] = in2_b[0:64]    # mean bias
    cst[0:64, 17] = wq_b * scale
    cst[0:64, 18] = in2_b[64:128]  # logstd bias
    cst[0:64, 19] = wk_b
    shared["cst"] = cst

    m = np.full((128, 128), -1e9, f)
    for b in range(16):
        blk = slice(b * 8, (b + 1) * 8)
        m[blk, blk] = 0.0
    np.fill_diagonal(m, -1e9)
    shared["mskd"] = np.ascontiguousarray(np.tile(m, (1, 4)), f)
    shared["eyed"] = np.eye(128, dtype=f)
    shared["onesd"] = np.ones((128, 128), f)

    la2 = np.asarray(last_actions, f).reshape(B * N, NA)
    in_maps = []
    for c in range(NCORES):
        rs = slice(c * R, (c + 1) * R)
        im = dict(shared)
        im["xin"] = np.ascontiguousarray(np.asarray(inputs, f)[rs].T)
        im["hin"] = np.ascontiguousarray(np.asarray(hidden_state, f)[rs].T)
        im["lat"] = np.ascontiguousarray(la2[rs].T)
        im["epst"] = np.ascontiguousarray(np.asarray(eps, f)[rs].T)
        in_maps.append(im)
    return in_maps


def kernel(inputs, last_actions, hidden_state, eps,
           fc1_w, fc1_b, gru_w_ih, gru_w_hh, gru_b_ih, gru_b_hh,
           in1_w, in1_b, bn_g, bn_b, in2_w, in2_b,
           wq_w, wq_b, wk_w, wk_b, wv_w, wv_b, fc2_w, fc2_b,
           bs, t):
    in_maps = _prep_host(
        inputs, last_actions, hidden_state, eps,
        fc1_w, fc1_b, gru_w_ih, gru_w_hh, gru_b_ih, gru_b_hh,
        in1_w, in1_b, bn_g, bn_b, in2_w, in2_b,
        wq_w, wq_b, wk_w, wk_b, wv_w, wv_b, fc2_w, fc2_b)

    nc = _get_nc(collective=True)
    res = run_bass_kernel_spmd(nc, in_maps, core_ids=list(range(NCORES)))

    f = np.float32
    h = np.empty((B * N, H), f)
    intent = np.empty((B * N, I), f)
    intent_embed = np.empty((B * N, 2 * I), f)
    local_q = np.empty((B * N, NA), f)
    for c in range(NCORES):
        r = res.results[c]
        rs = slice(c * R, (c + 1) * R)
        h[rs] = r["hT_o"].T
        intent[rs] = r["intT_o"].T
        intent_embed[rs] = r["ipT_o"].T
        local_q[rs] = r["lq_o"]
    return local_q.reshape(B, N, NA), h, intent, intent_embed
